# revision 9
# baseline (speedup 1.0000x reference)
"""Trainium2 Bass kernel for nn_ClusteringLoss.

Reference computation (see problem statement):
    pred   = predicted_distribution[0]            # [N, K]
    labels = argmax(pred, -1)                     # [N]
    S      = +1/-1 agreement matrix [N, N]
    M      = (target == 1)                        # [B, N, K]
    n      = M.sum(1)                             # [B, K]
    quad   = einsum('bnk,nm,bmk->bk', M, S, M)
    loss   = ((quad - n)/2).sum() / (n(n-1)/2).sum()

Algebraic reduction: with E = onehot(argmax(pred)) [N, L=K],
S = 2 E E^T - 1, so with the count matrix C[b] = E^T M[b]  ([L, K]):
    quad[b,k] = 2 * sum_l C[b,l,k]^2 - n[b,k]^2,   n[b,k] = sum_l C[b,l,k]
    loss_num  = sum_{b,k} ( sum_l C^2 - n(n+1)/2 )
    loss_den  = sum_{b,k} n(n-1)/2
So each core only needs to produce C[b] (a [32, 32] f32 count matrix);
the host finishes the (tiny) scalar reduction.

Sharding: data-parallel over B=8 (one event per NeuronCore). Every core
receives pred[0] (replicated) + its own target[b].

Device kernel per core — raw Bass (no Tile framework: avoids the Tile
end-of-kernel drain + EVSEM-butterfly tail, ~8 us), manual semaphores,
two pipeline halves:
    SP  ring: DMA pred half 0/1          (HWDGE qSPDynamicHW)
    ACT ring: DMA tgt  half 0/1          (HWDGE qActDynamicHW, parallel issue)
    DVE:  per half: rowmax (reduce max) + is_equal -> one-hot E (bf16)
    POOL: per half: tgt f32 -> bf16 convert (idle engine, no ACT table load)
    PE:   per half: 16 accumulating matmuls E_g^T @ M_g into PSUM C [32,32]
    DVE:  C -> SBUF;  SP: C -> DRAM.
E/M are 0/1 so bf16 matmul products are exact; PSUM accumulates fp32
(exact integer counts). The one-hot uses plain is_equal-vs-rowmax: valid
when no row has two bit-identical f32 maxima, which holds for this input
distribution (verified for the fixed seed; measure-zero event for randn).
"""

import numpy as np

try:
    import concourse.bass as bass  # noqa: F401
except ImportError:  # harness may run from a bare directory
    import sys

    sys.path.insert(0, "/opt/trn_rl_repo")

import concourse.bass as bass
import concourse.mybir as mybir
from concourse.bass_utils import run_bass_kernel_spmd

B, N, K = 8, 4096, 32
P = 128          # SBUF partitions
G = N // P       # 32 row-groups per partition
H = G // 2       # groups per pipeline half
FP32 = mybir.dt.float32
BF16 = mybir.dt.bfloat16

_CACHE = {}


def _build_nc():
    nc = bass.Bass("TRN2", target_bir_lowering=False, debug=False)
    pred0 = nc.dram_tensor("pred0", [N, K], FP32, kind="ExternalInput").ap()
    tgt = nc.dram_tensor("tgt", [N, K], FP32, kind="ExternalInput").ap()
    outc = nc.dram_tensor("outc", [K, K], FP32, kind="ExternalOutput").ap()

    pred_r = pred0.rearrange("(p g) k -> p g k", p=P)
    tgt_r = tgt.rearrange("(p g) k -> p g k", p=P)

    with (
        nc.sbuf_tensor("pred_sb", [P, G, K], FP32) as pred_sb_h,
        nc.sbuf_tensor("tgt_sb", [P, G, K], FP32) as tgt_sb_h,
        nc.sbuf_tensor("rowmax", [P, G], FP32) as rowmax_h,
        nc.sbuf_tensor("eqb", [P, G, K], BF16) as eqb_h,
        nc.sbuf_tensor("tgtb", [P, G, K], BF16) as tgtb_h,
        nc.sbuf_tensor("csb", [K, K], FP32) as csb_h,
        nc.psum_tensor("psumc", [K, K], FP32) as psumc_h,
        nc.semaphore("s_pred") as s_pred,
        nc.semaphore("s_tgt") as s_tgt,
        nc.semaphore("s_eq") as s_eq,
        nc.semaphore("s_tgtb") as s_tgtb,
        nc.semaphore("s_mm") as s_mm,
        nc.semaphore("s_csb") as s_csb,
        nc.semaphore("s_out") as s_out,
        nc.Block() as block,
    ):
        pred_sb = pred_sb_h.ap()
        tgt_sb = tgt_sb_h.ap()
        rowmax = rowmax_h.ap()
        eqb = eqb_h.ap()
        tgtb = tgtb_h.ap()
        csb = csb_h.ap()
        psumc = psumc_h.ap()
        halves = [slice(0, H), slice(H, G)]

        @block.sync
        def _(sync):
            for hs in halves:
                sync.dma_start(pred_sb[:, hs, :], pred_r[:, hs, :]).then_inc(
                    s_pred, 16
                )
            sync.wait_ge(s_csb, 1)
            sync.dma_start(outc, csb).then_inc(s_out, 16)
            sync.wait_ge(s_out, 16)

        @block.scalar
        def _(scalar):
            for hs in halves:
                scalar.dma_start(tgt_sb[:, hs, :], tgt_r[:, hs, :]).then_inc(
                    s_tgt, 16
                )

        @block.gpsimd
        def _(gpsimd):
            for h, hs in enumerate(halves):
                gpsimd.wait_ge(s_tgt, 16 * (h + 1))
                gpsimd.tensor_copy(tgtb[:, hs, :], tgt_sb[:, hs, :]).then_inc(
                    s_tgtb, 1
                )

        @block.vector
        def _(vector):
            for h, hs in enumerate(halves):
                vector.wait_ge(s_pred, 16 * (h + 1))
                vector.tensor_reduce(
                    rowmax[:, hs],
                    pred_sb[:, hs, :],
                    axis=mybir.AxisListType.X,
                    op=mybir.AluOpType.max,
                )
                vector.tensor_tensor(
                    eqb[:, hs, :],
                    pred_sb[:, hs, :],
                    rowmax[:, hs, None].broadcast_to([P, H, K]),
                    op=mybir.AluOpType.is_equal,
                ).then_inc(s_eq, 1)
            vector.wait_ge(s_mm, 1)
            vector.tensor_copy(csb, psumc).then_inc(s_csb, 1)

        @block.tensor
        def _(tensor):
            for h in range(2):
                tensor.wait_ge(s_eq, h + 1)
                tensor.wait_ge(s_tgtb, h + 1)
                for gi in range(H):
                    g = h * H + gi
                    mm = tensor.matmul(
                        psumc,
                        eqb[:, g, :],
                        tgtb[:, g, :],
                        start=(g == 0),
                        stop=(g == G - 1),
                    )
            mm.then_inc(s_mm, 1)

    return nc


def _get_nc():
    if "nc" not in _CACHE:
        _CACHE["nc"] = _build_nc()
    return _CACHE["nc"]


def _finish(cs):
    """Host-side scalar reduction from the 8 per-core count matrices."""
    s1 = s2 = s3 = 0.0
    for C in cs:
        C = C.astype(np.float64)
        n = C.sum(axis=0)
        s1 += (C * C).sum()
        s2 += (n * n).sum()
        s3 += n.sum()
    loss = s1 - 0.5 * (s2 + s3)
    comparisons = 0.5 * (s2 - s3)
    return np.asarray(np.float32(loss / comparisons))


def kernel(predicted_distribution, target_distribution, _trace=False, **_kw):
    nc = _get_nc()
    pred0 = np.ascontiguousarray(predicted_distribution[0], dtype=np.float32)
    in_maps = [
        {
            "pred0": pred0,
            "tgt": np.ascontiguousarray(target_distribution[b], dtype=np.float32),
        }
        for b in range(B)
    ]
    res = run_bass_kernel_spmd(nc, in_maps, core_ids=list(range(B)), trace=_trace)
    if _trace:
        _CACHE["last_results"] = res
    return _finish([r["outc"] for r in res.results])
